# revision 1
# baseline (speedup 1.0000x reference)
"""Trainium2 Bass kernel for nn_EqPBC (triplet-feature PBC equalizer).

Data-parallel over 8 NeuronCores: each core handles 8192 samples.
Per core, per chunk of 512 samples (batch on free dim, features on partitions):
  1. DMA [128,82] f32 blocks, cast bf16 (DVE), PE-transpose -> E^T [82,512] bf16
  2. One-hot gather matmuls (PE): En/Em/Emn rows (p,h) split (p, h<128|h>=128)
  3. DVE: S1 = sum_p En_p*conj(Emn_p), S2 = sum_p Em_p*conj(Emn_p),
     X_i = Em_i*S1 + En_i*S2  (complex, bf16)
  4. PE reduction over h with W' = W[i,h]*(0.5 on diag) folded into lhsT
  5. f32 finish: out = E[:,L,:] + Eout * 10^(task0/10)/2  (exact f32 E_L term)

Out-of-bounds Emn indices replicate JAX gather semantics: wrap negatives,
then clamp -> both OOB entries land on index 40.
"""
import numpy as np
import ml_dtypes
from contextlib import ExitStack

# ----- static problem constants (hardcoded; kernel.py must be self-contained) -----
M = 41
L = M // 2
NMODES = 2
B = 65536
NCORES = 8
BC = B // NCORES          # 8192 samples per core
NB = 512                  # samples per chunk
NCHUNK = BC // NB         # 16
THRESH = 1.0 * M // 2
_idx = [(m, n) for m in range(-L, L + 1) for n in range(m, L + 1) if abs(m * n) <= THRESH]
M_ARR = np.array([m for m, n in _idx], dtype=np.int32)
N_ARR = np.array([n for m, n in _idx], dtype=np.int32)
DIAG = np.array([m == n for m, n in _idx])
HDIM = len(_idx)          # 177
HA = 128                  # h-split: a block [0,128), b block [128,177)
HB = HDIM - HA            # 49

bf16 = ml_dtypes.bfloat16


def _gather_cols(idx_arr):
    """Column indices into E^T[82,:] (row f = 2*m + p) for gathered rows (p,h)."""
    src = np.empty((2, HDIM), dtype=np.int64)
    for p in range(2):
        src[p] = 2 * (L + idx_arr) + p
    return src  # [p, h] -> source row in [0,82)


BP = 113   # packed b-block rows: p0 tail at 0:49, p1 tail at 64:113
GCOLS = 128 + BP + 128  # 369


def _build_consts():
    mn = L + M_ARR + N_ARR
    mn = np.clip(np.where(mn < 0, mn + M, mn), 0, M - 1) - L  # jax wrap+clamp
    srcs = {"n": _gather_cols(N_ARR), "m": _gather_cols(M_ARR), "mn": _gather_cols(mn)}
    gmats = {}
    for k, src in srcs.items():
        G = np.zeros((82, GCOLS), dtype=np.float32)
        for p in range(2):
            for h in range(HA):                      # a-blocks
                G[src[p, h], (0 if p == 0 else 128 + BP) + h] = 1.0
            for r in range(HB):                      # packed b-block
                G[src[p, HA + r], 128 + (0 if p == 0 else 64) + r] = 1.0
        gmats[k] = G.astype(bf16)
    return gmats


def _build_wred(Wr, Wi):
    """[177, 8] bf16: cols (i*4+0,1) = (W'r,W'i) for rhs=X_ir;
    cols (i*4+2,3) = (-W'i, W'r) for rhs=X_ii.  W' = W[i]*(0.5 on diag)."""
    scale = np.where(DIAG, 0.5, 1.0).astype(np.float32)
    out = np.zeros((HDIM, 8), dtype=np.float32)
    for i in range(2):
        wr = Wr[i] * scale
        wi = Wi[i] * scale
        out[:, i * 4 + 0] = wr
        out[:, i * 4 + 1] = wi
        out[:, i * 4 + 2] = -wi
        out[:, i * 4 + 3] = wr
    return out.astype(bf16)


def _build_ffold():
    """[113,113] bf16: out[j] = in[j] + in[64+j] for j<49, replicated at 64+j;
    pad cols 49:64 zero."""
    F = np.zeros((BP, BP), dtype=np.float32)
    for h in range(HB):
        for r in (h, 64 + h):
            F[r, h] = 1.0
            F[r, 64 + h] = 1.0
    return F.astype(bf16)


def _build_wredb(Wr, Wi):
    """[113, 8] bf16 for the packed b-block: mode0 tail at rows 0:49,
    mode1 tail at rows 64:113; col layout as in _build_wred."""
    scale = np.where(DIAG, 0.5, 1.0).astype(np.float32)
    out = np.zeros((BP, 8), dtype=np.float32)
    for i in range(2):
        wr = (Wr[i] * scale)[HA:]
        wi = (Wi[i] * scale)[HA:]
        r0 = 0 if i == 0 else 64
        out[r0:r0 + HB, i * 4 + 0] = wr
        out[r0:r0 + HB, i * 4 + 1] = wi
        out[r0:r0 + HB, i * 4 + 2] = -wi
        out[r0:r0 + HB, i * 4 + 3] = wr
    return out.astype(bf16)


def _build_kernel():
    import concourse.bass as bass
    import concourse.bacc as bacc
    import concourse.tile as tile
    import concourse.mybir as mybir

    dt = mybir.dt
    nc = bacc.Bacc("TRN2", target_bir_lowering=False, debug=False, num_devices=NCORES)
    xr = nc.declare_dram_parameter("xr", [BC, 82], dt.float32, isOutput=False)
    xi = nc.declare_dram_parameter("xi", [BC, 82], dt.float32, isOutput=False)
    ti = nc.declare_dram_parameter("ti", [BC, 4], dt.float32, isOutput=False)
    gn_d = nc.declare_dram_parameter("gn", [82, GCOLS], dt.bfloat16, isOutput=False)
    gm_d = nc.declare_dram_parameter("gm", [82, GCOLS], dt.bfloat16, isOutput=False)
    gmn_d = nc.declare_dram_parameter("gmn", [82, GCOLS], dt.bfloat16, isOutput=False)
    wred_d = nc.declare_dram_parameter("wred", [HDIM, 8], dt.bfloat16, isOutput=False)
    wredb_d = nc.declare_dram_parameter("wredb", [BP, 8], dt.bfloat16, isOutput=False)
    ffold_d = nc.declare_dram_parameter("ffold", [BP, BP], dt.bfloat16, isOutput=False)
    id128_d = nc.declare_dram_parameter("id128", [128, 128], dt.float32, isOutput=False)
    id4_d = nc.declare_dram_parameter("id4", [2, 2], dt.float32, isOutput=False)
    out_d = nc.declare_dram_parameter("out", [BC, 4], dt.float32, isOutput=True)

    LN10_10 = float(np.log(10.0) / 10.0)
    LNHALF = float(np.log(0.5))

    with tile.TileContext(nc) as tc, ExitStack() as ctx:
        cpool = ctx.enter_context(tc.tile_pool(name="consts", bufs=1))
        nat_pool = ctx.enter_context(tc.tile_pool(name="nat", bufs=6))
        et_pool = ctx.enter_context(tc.tile_pool(name="et", bufs=3))
        g_pool = ctx.enter_context(tc.tile_pool(name="gath", bufs=2))
        s_pool = ctx.enter_context(tc.tile_pool(name="smid", bufs=3))
        tmp_pool = ctx.enter_context(tc.tile_pool(name="tmps", bufs=2))
        x_pool = ctx.enter_context(tc.tile_pool(name="xmid", bufs=3))
        e_pool = ctx.enter_context(tc.tile_pool(name="eall", bufs=2))
        o_pool = ctx.enter_context(tc.tile_pool(name="outs", bufs=2))
        pt_psum = ctx.enter_context(tc.tile_pool(name="ptp", bufs=1, space="PSUM"))
        pg_psum = ctx.enter_context(tc.tile_pool(name="pgp", bufs=4, space="PSUM"))
        pe_psum = ctx.enter_context(tc.tile_pool(name="pep", bufs=2, space="PSUM"))
        po_psum = ctx.enter_context(tc.tile_pool(name="pop", bufs=1, space="PSUM"))

        # load constants once
        gmats_sb = {}
        for name, d in (("n", gn_d), ("m", gm_d), ("mn", gmn_d)):
            t = cpool.tile([82, GCOLS], dt.bfloat16, tag=f"g{name}")
            nc.gpsimd.dma_start(out=t[:], in_=d[:])
            gmats_sb[name] = t
        wredA = cpool.tile([HA, 8], dt.bfloat16, tag="wredA")
        nc.gpsimd.dma_start(out=wredA[:], in_=wred_d[0:HA, :])
        wredB = cpool.tile([BP, 8], dt.bfloat16, tag="wredB")
        nc.gpsimd.dma_start(out=wredB[:], in_=wredb_d[:])
        id128 = cpool.tile([128, 128], dt.float32, tag="id128")
        nc.gpsimd.dma_start(out=id128[:], in_=id128_d[:])
        id4 = cpool.tile([2, 2], dt.float32, tag="id4")
        nc.gpsimd.dma_start(out=id4[:], in_=id4_d[:])
        ffold = cpool.tile([BP, BP], dt.bfloat16, tag="ffold")
        nc.gpsimd.dma_start(out=ffold[:], in_=ffold_d[:])
        bias_t = cpool.tile([128, 1], dt.float32, tag="biasln")
        nc.vector.memset(bias_t[:], LNHALF)

        # J-slices of gather matrices: [a0(p0 h<128), bpack(113), a1(p1 h<128)]
        jslices = [(0, HA), (HA, BP), (HA + BP, HA)]

        for c in range(NCHUNK):
            b0 = c * NB
            nat = {}
            etT = {}
            for comp, src in (("r", xr), ("i", xi)):
                et = et_pool.tile([82, NB], dt.bfloat16, tag=f"et{comp}")
                etT[comp] = et
                for blk in range(4):
                    t = nat_pool.tile([128, 82], dt.float32, tag=f"nat{comp}")
                    nc.gpsimd.dma_start(out=t[:], in_=src[b0 + blk * 128: b0 + (blk + 1) * 128, :])
                    if blk == 3:
                        nat[comp] = t  # keep last block for E_L columns (see below)
                    nat[(comp, blk)] = t
                    pt = pt_psum.tile([82, 128], dt.float32, tag="tpsum")
                    nc.tensor.transpose(pt[:], t[:], id128[:])
                    nc.scalar.copy(et[:, blk * 128:(blk + 1) * 128], pt[:])

            # gathers: gtile[kind][comp][j] with j in 0..3 = (p0a,p0b,p1a,p1b)
            gt = {}
            for kind in ("n", "m", "mn"):
                for comp in ("r", "i"):
                    for j, (j0, jl) in enumerate(jslices):
                        ps = pg_psum.tile([128, NB], dt.float32, tag="gpsum")
                        nc.tensor.matmul(ps[:jl, :], gmats_sb[kind][:, j0:j0 + jl],
                                         etT[comp][:], start=True, stop=True)
                        sb = g_pool.tile([128, NB], dt.bfloat16, tag=f"g{kind}{comp}{j}")
                        nc.scalar.copy(sb[:jl, :], ps[:jl, :])
                        gt[(kind, comp, j)] = sb

            def TT(op, out, a, b_, rows, eng=None):
                getattr(eng or nc.vector, op)(out[:rows, :], a[:rows, :], b_[:rows, :])

            # S-stage over 3 gathered tiles: j=0 a0(p0,128), j=1 bpack(113: p0
            # tail at 0:49, p1 tail at 64:113), j=2 a1(p1,128).
            S = {}
            for (sname, kind) in (("S1", "n"), ("S2", "m")):
                for comp in ("r", "i"):
                    prods = {}
                    for j, rows in ((0, HA), (1, BP), (2, HA)):
                        pa = tmp_pool.tile([128, NB], dt.bfloat16, tag=f"pa{j}")
                        pb_ = tmp_pool.tile([128, NB], dt.bfloat16, tag=f"pb{j}")
                        if comp == "r":
                            TT("tensor_mul", pa, gt[(kind, "r", j)], gt[("mn", "r", j)], rows)
                            TT("tensor_mul", pb_, gt[(kind, "i", j)], gt[("mn", "i", j)], rows)
                        else:
                            TT("tensor_mul", pa, gt[(kind, "i", j)], gt[("mn", "r", j)], rows)
                            TT("tensor_mul", pb_, gt[(kind, "r", j)], gt[("mn", "i", j)], rows)
                        prods[j] = (pa, pb_)
                    qs = {}
                    for j, rows in ((0, HA), (1, BP), (2, HA)):
                        q = tmp_pool.tile([128, NB], dt.bfloat16, tag=f"q{j}")
                        TT("tensor_add" if comp == "r" else "tensor_sub", q, prods[j][0], prods[j][1], rows)
                        qs[j] = q
                    sa = s_pool.tile([128, NB], dt.bfloat16, tag=f"{sname}{comp}a")
                    TT("tensor_add", sa, qs[0], qs[2], HA)
                    # packed b fold: S_b[r] = q1[r] + q1[64+r]; write it at BOTH
                    # row offsets so it aligns with either mode's packed operand
                    psf = pg_psum.tile([128, NB], dt.float32, tag="gpsum")
                    nc.tensor.matmul(psf[:BP, :], ffold[:], qs[1][:BP, :], start=True, stop=True)
                    sbp = s_pool.tile([128, NB], dt.bfloat16, tag=f"{sname}{comp}b")
                    nc.scalar.copy(sbp[:BP, :], psf[:BP, :])
                    S[(sname, comp, 0)] = sa
                    S[(sname, comp, 1)] = sbp

            # X-stage: a-blocks per mode i (gt j = 0 or 2); b-block packed for
            # both modes at once (gt j = 1, lhsT weights select the mode rows).
            X = {}
            for comp in ("r", "i"):
                s1a, s1b = "S1", "S2"
                ops = []  # (out_key, gkind_tiles_j, rows)
                for sel in (0, 2, 1):  # a0 (i=0), a1 (i=1), bpack (both)
                    rows = BP if sel == 1 else HA
                    hb = 1 if sel == 1 else 0
                    t1 = tmp_pool.tile([128, NB], dt.bfloat16, tag=f"xt1{sel}")
                    t2 = tmp_pool.tile([128, NB], dt.bfloat16, tag=f"xt2{sel}")
                    t3 = tmp_pool.tile([128, NB], dt.bfloat16, tag=f"xt3{sel}")
                    t4 = tmp_pool.tile([128, NB], dt.bfloat16, tag=f"xt4{sel}")
                    if comp == "r":
                        TT("tensor_mul", t1, gt[("m", "r", sel)], S[("S1", "r", hb)], rows)
                        TT("tensor_mul", t2, gt[("m", "i", sel)], S[("S1", "i", hb)], rows)
                        TT("tensor_mul", t3, gt[("n", "r", sel)], S[("S2", "r", hb)], rows)
                        TT("tensor_mul", t4, gt[("n", "i", sel)], S[("S2", "i", hb)], rows)
                    else:
                        TT("tensor_mul", t1, gt[("m", "r", sel)], S[("S1", "i", hb)], rows)
                        TT("tensor_mul", t2, gt[("m", "i", sel)], S[("S1", "r", hb)], rows)
                        TT("tensor_mul", t3, gt[("n", "r", sel)], S[("S2", "i", hb)], rows)
                        TT("tensor_mul", t4, gt[("n", "i", sel)], S[("S2", "r", hb)], rows)
                    ops.append((sel, rows, t1, t2, t3, t4))
                uv = {}
                for sel, rows, t1, t2, t3, t4 in ops:
                    u = tmp_pool.tile([128, NB], dt.bfloat16, tag=f"xu{sel}")
                    v = tmp_pool.tile([128, NB], dt.bfloat16, tag=f"xv{sel}")
                    if comp == "r":
                        TT("tensor_sub", u, t1, t2, rows)
                        TT("tensor_sub", v, t3, t4, rows)
                    else:
                        TT("tensor_add", u, t1, t2, rows)
                        TT("tensor_add", v, t3, t4, rows)
                    uv[sel] = (u, v, rows)
                for sel in (0, 2, 1):
                    u, v, rows = uv[sel]
                    xt = x_pool.tile([128, NB], dt.bfloat16, tag=f"x{comp}{sel}")
                    TT("tensor_add", xt, u, v, rows)
                    X[(comp, sel)] = xt

            # reduction: Eout_i = sum_h W'_i[h] * X_i[h]; a-block per mode
            # (X[(comp, 0|2)]) + packed-b (X[(comp,1)], wredB rows select mode)
            eall0 = e_pool.tile([2, NB], dt.float32, tag="eall0")
            eall1 = e_pool.tile([2, NB], dt.float32, tag="eall1")
            eall = [eall0, eall1]
            for i in range(2):
                sel = 0 if i == 0 else 2
                pe = pe_psum.tile([2, NB], dt.float32, tag="epsum")
                nc.tensor.matmul(pe[:], wredA[:, i * 4:i * 4 + 2], X[("r", sel)][:HA, :],
                                 start=True, stop=False)
                nc.tensor.matmul(pe[:], wredA[:, i * 4 + 2:i * 4 + 4], X[("i", sel)][:HA, :],
                                 start=False, stop=False)
                nc.tensor.matmul(pe[:], wredB[:, i * 4:i * 4 + 2], X[("r", 1)][:BP, :],
                                 start=False, stop=False)
                nc.tensor.matmul(pe[:], wredB[:, i * 4 + 2:i * 4 + 4], X[("i", 1)][:BP, :],
                                 start=False, stop=True)
                nc.scalar.copy(eall[i][:], pe[:])

            # final combine per 128-block
            for blk in range(4):
                po = po_psum.tile([128, 4], dt.float32, tag="opsum")
                nc.tensor.transpose(po[:, 0:2], eall[0][:, blk * 128:(blk + 1) * 128], id4[:])
                nc.tensor.transpose(po[:, 2:4], eall[1][:, blk * 128:(blk + 1) * 128], id4[:])
                tit = o_pool.tile([128, 4], dt.float32, tag="tit")
                nc.gpsimd.dma_start(out=tit[:], in_=ti[b0 + blk * 128: b0 + (blk + 1) * 128, :])
                pcol = o_pool.tile([128, 1], dt.float32, tag="pcol")
                import concourse.mybir as _mb
                nc.scalar.activation(pcol[:], tit[:, 0:1], _mb.ActivationFunctionType.Exp,
                                     bias=bias_t[:], scale=LN10_10)
                ot = o_pool.tile([128, 4], dt.float32, tag="ot")
                nc.vector.tensor_scalar_mul(ot[:], po[:], pcol[:])
                # add exact E_L columns: out cols (0,2) += xr_nat[:, 40:42]; (1,3) += xi_nat
                nc.vector.tensor_add(ot[:, 0:4:2], ot[:, 0:4:2], nat[("r", blk)][:, 2 * L:2 * L + 2])
                nc.vector.tensor_add(ot[:, 1:4:2], ot[:, 1:4:2], nat[("i", blk)][:, 2 * L:2 * L + 2])
                nc.sync.dma_start(out=out_d[b0 + blk * 128: b0 + (blk + 1) * 128, :], in_=ot[:])

    nc.compile()
    return nc


_CACHE = {}


def kernel(xr, xi, task_info, Wr, Wi):
    from concourse.bass_utils import run_bass_kernel_spmd

    xr = np.ascontiguousarray(np.asarray(xr, dtype=np.float32)).reshape(B, 82)
    xi = np.ascontiguousarray(np.asarray(xi, dtype=np.float32)).reshape(B, 82)
    task_info = np.ascontiguousarray(np.asarray(task_info, dtype=np.float32))
    gm = _build_consts()
    Wr32 = np.asarray(Wr, dtype=np.float32); Wi32 = np.asarray(Wi, dtype=np.float32)
    wred = _build_wred(Wr32, Wi32)
    wredb = _build_wredb(Wr32, Wi32)
    id128 = np.eye(128, dtype=np.float32)
    id4 = np.eye(2, dtype=np.float32)

    if "nc" not in _CACHE:
        _CACHE["nc"] = _build_kernel()
    nc = _CACHE["nc"]

    in_maps = []
    for core in range(NCORES):
        s = slice(core * BC, (core + 1) * BC)
        in_maps.append({
            "xr": xr[s], "xi": xi[s], "ti": task_info[s],
            "gn": gm["n"], "gm": gm["m"], "gmn": gm["mn"],
            "wred": wred, "wredb": wredb, "ffold": _build_ffold(), "id128": id128, "id4": id4,
        })
    res = run_bass_kernel_spmd(nc, in_maps, list(range(NCORES)))
    outs = [res.results[i]["out"] for i in range(NCORES)]
    full = np.concatenate(outs, axis=0)  # [B, 4]
    return full.reshape(B, NMODES, 2).astype(np.float32)



# revision 16
# speedup vs baseline: 1.9169x; 1.9169x over previous
"""Trainium2 Bass kernel for nn_EqPBC (triplet-feature PBC equalizer).

Correlation formulation: for each sample, every triplet feature is a
product of correlation values C(a,b) = sum_p E_a,p * conj(E_b,p) with the
gathered E rows, and the weighted h-reduction is linear in C.  So:

  1. E^T tiles [105, NB] per chunk (p0 rows 0:41, p1 rows 64:105) via PE
     transposes of host-prepermuted [128, 8*105] blocks.
  2. Gather A/B operand stacks for the 221 distinct non-diagonal pairs
     (2 row-blocks) with p packed along free dim -> 4 DVE muls + 2 adds
     (+2 optional p-folds) per block; diagonal C(a,a) = |E_a|^2 comes
     straight off E^T (2 muls + 1 add), p-fold folded into T weights.
  3. T_{k,i} = sum_h W'_ih S{1,2}_h grouped by shift row k: constant
     lhsT matmuls over the C tiles, PSUM-accumulated (all adds on PE).
  4. X = E o T (4 DVE muls), masked-reduce matmuls -> Eout [4, NB].
  5. finish: out = E_L (exact f32) + Eout * 10^(ti0/10)/2.
"""
import numpy as np
import ml_dtypes
from contextlib import ExitStack

# ----- static problem constants (hardcoded; kernel.py must be self-contained) -----
M = 41
L = M // 2
NMODES = 2
B = 65536
NCORES = 8
BC = B // NCORES          # 8192 samples per core
NB = 1024                 # samples per chunk
NBLK = NB // 128          # 8 natural sample blocks per chunk
NCHUNK = BC // NB         # 8
W105 = 105                # et rows: p0 at 0:41, p1 at 64:105
THRESH = 1.0 * M // 2

bf16 = ml_dtypes.bfloat16

_idx = [(m, n) for m in range(-L, L + 1) for n in range(m, L + 1) if abs(m * n) <= THRESH]
HDIM = len(_idx)          # 177


def _fix(k):
    """jax gather semantics for row L+k: wrap negatives, then clamp."""
    r = L + k
    if r < 0:
        r += M
    return min(max(r, 0), M - 1)


def _terms():
    """(ktgt_row, a, b, sigma, h): Eout_i += W'_ih * C~(a,b) * E_{ktgt,i},
    C~ = C if sigma=+1 else conj(C); two terms per h (S1 and S2)."""
    out = []
    for h, (m, n) in enumerate(_idx):
        kmn = _fix(m + n)
        kn = L + n
        km = L + m
        for (ktgt, a, b) in ((km, kn, kmn), (kn, km, kmn)):
            if a <= b:
                out.append((ktgt, a, b, 1, h))
            else:
                out.append((ktgt, b, a, -1, h))
    return out


TERMS = _terms()
NONDIAG = sorted({(a, b) for (_, a, b, _, _) in TERMS if a != b})
NPAIR = len(NONDIAG)      # 221
PIDX = {p: i for i, p in enumerate(NONDIAG)}
BL0 = 128
BL1 = NPAIR - BL0         # 93
BLOCKS = ((0, BL0), (BL0, BL1))


def _build_gmats():
    """Gather lhsT [105, rows] per (stack A/B, block, p): selects et row
    64*p + a (or b) of each pair into the stack's pair-row."""
    gs = {}
    for blk, (j0, jl) in enumerate(BLOCKS):
        for p in range(2):
            ga = np.zeros((W105, jl), np.float32)
            gb = np.zeros((W105, jl), np.float32)
            for j in range(jl):
                a, b = NONDIAG[j0 + j]
                ga[64 * p + a, j] = 1.0
                gb[64 * p + b, j] = 1.0
            gs[("A", blk, p)] = ga.astype(bf16)
            gs[("B", blk, p)] = gb.astype(bf16)
    return gs


def _build_tw(Wr, Wi):
    """T-stage lhsT weights: rhs in {Cr0, Cr1, Ci0, Ci1, D0} x Tcomp {r,i},
    shape [rhs_rows, 105]; col = 64*i + ktgt.  W' = W * (0.5 on diag h)."""
    scale = np.array([1.0 if m != n else 0.5 for (m, n) in _idx], np.float32)
    wr = Wr * scale
    wi = Wi * scale
    shapes = {}
    for blk, jl in ((0, BL0), (1, BL1)):
        for tcomp in ("r", "i"):
            shapes[("Cr", blk, tcomp)] = (jl, W105)
            shapes[("Ci", blk, tcomp)] = (jl, W105)
    shapes[("D0", "r")] = (W105, W105)
    shapes[("D0", "i")] = (W105, W105)
    mats = {k: np.zeros(s, np.float32) for k, s in shapes.items()}
    for (ktgt, a, b, sig, h) in TERMS:
        for i in range(2):
            col = 64 * i + ktgt
            if a == b:
                # rhs d0pre rows 64*p + a hold |E_{a,p}|^2; p-fold via both rows
                for row in (a, 64 + a):
                    mats[("D0", "r")][row, col] += wr[i, h]
                    mats[("D0", "i")][row, col] += wi[i, h]
            else:
                j = PIDX[(a, b)]
                blk, r = (0, j) if j < BL0 else (1, j - BL0)
                mats[("Cr", blk, "r")][r, col] += wr[i, h]
                mats[("Ci", blk, "r")][r, col] += -sig * wi[i, h]
                mats[("Cr", blk, "i")][r, col] += wi[i, h]
                mats[("Ci", blk, "i")][r, col] += sig * wr[i, h]
    return {k: v.astype(bf16) for k, v in mats.items()}


def _build_masks():
    """Reduce lhsT [105, 4]: Eout rows (re0, im0, re1, im1) from the four
    X products P1=Er*Tr, P2=Ei*Ti, P3=Er*Ti, P4=Ei*Tr."""
    ms = {}
    for name, sgn, cc in (("P1", 1, 0), ("P2", -1, 0), ("P3", 1, 1), ("P4", 1, 1)):
        mm_ = np.zeros((W105, 4), np.float32)
        for i in range(2):
            for k in range(M):
                mm_[64 * i + k, 2 * i + cc] = sgn
        ms[name] = mm_.astype(bf16)
    return ms


def _build_kernel():
    import concourse.bass as bass
    import concourse.bacc as bacc
    import concourse.tile as tile
    import concourse.mybir as mybir

    dt = mybir.dt
    nc = bacc.Bacc("TRN2", target_bir_lowering=False, debug=False, num_devices=NCORES)
    # host-prepermuted inputs: row r = chunk*NB + p*NBLK + blk, cols 0:41 p0 /
    # 64:105 p1 (cols 41:64 zero)
    xhr = nc.declare_dram_parameter("xhr", [BC, W105], dt.float32, isOutput=False)
    xhi = nc.declare_dram_parameter("xhi", [BC, W105], dt.float32, isOutput=False)
    tih = nc.declare_dram_parameter("tih", [BC, 4], dt.float32, isOutput=False)
    gm_d = {}
    for key in (("A", 0, 0), ("A", 0, 1), ("A", 1, 0), ("A", 1, 1),
                ("B", 0, 0), ("B", 0, 1), ("B", 1, 0), ("B", 1, 1)):
        st, blk, p = key
        jl = BLOCKS[blk][1]
        gm_d[key] = nc.declare_dram_parameter(f"g{st}{blk}{p}", [W105, jl], dt.bfloat16, isOutput=False)
    tw_d = {}
    for key, shape in (
        (("Cr", 0, "r"), (BL0, W105)), (("Cr", 1, "r"), (BL1, W105)),
        (("Ci", 0, "r"), (BL0, W105)), (("Ci", 1, "r"), (BL1, W105)),
        (("Cr", 0, "i"), (BL0, W105)), (("Cr", 1, "i"), (BL1, W105)),
        (("Ci", 0, "i"), (BL0, W105)), (("Ci", 1, "i"), (BL1, W105)),
        (("D0", "r"), (W105, W105)), (("D0", "i"), (W105, W105)),
    ):
        nm = "tw" + "".join(str(x) for x in key)
        tw_d[key] = nc.declare_dram_parameter(nm, list(shape), dt.bfloat16, isOutput=False)
    mk_d = {}
    for name in ("P1", "P2", "P3", "P4"):
        mk_d[name] = nc.declare_dram_parameter(f"mk{name}", [W105, 4], dt.bfloat16, isOutput=False)
    id128_d = nc.declare_dram_parameter("id128", [128, 128], dt.float32, isOutput=False)
    id4_d = nc.declare_dram_parameter("id4", [4, 4], dt.float32, isOutput=False)
    out_d = nc.declare_dram_parameter("out", [BC, 4], dt.float32, isOutput=True)

    LN10_10 = float(np.log(10.0) / 10.0)
    LNHALF = float(np.log(0.5))

    with tile.TileContext(nc) as tc, ExitStack() as ctx:
        cpool = ctx.enter_context(tc.tile_pool(name="consts", bufs=1))
        natp = ctx.enter_context(tc.tile_pool(name="natp", bufs=2))
        etp = ctx.enter_context(tc.tile_pool(name="etp", bufs=2))
        stp = ctx.enter_context(tc.tile_pool(name="stp", bufs=2))
        sqp = ctx.enter_context(tc.tile_pool(name="sqp", bufs=2))
        prodp = ctx.enter_context(tc.tile_pool(name="prodp", bufs=2))
        prep = ctx.enter_context(tc.tile_pool(name="prep", bufs=2))
        cp = ctx.enter_context(tc.tile_pool(name="cp", bufs=4))
        tp = ctx.enter_context(tc.tile_pool(name="tp", bufs=2))
        xp = ctx.enter_context(tc.tile_pool(name="xp", bufs=2))
        ep = ctx.enter_context(tc.tile_pool(name="ep", bufs=2))
        finp = ctx.enter_context(tc.tile_pool(name="finp", bufs=2))
        # PSUM (16KB/partition): wps tag "w" [105,NB]f32 x2 bufs (8KB) +
        # sps tag "stps" [128,NB]f32 x2 bufs (8KB)
        wps = ctx.enter_context(tc.tile_pool(name="wps", bufs=2, space="PSUM"))
        sps = ctx.enter_context(tc.tile_pool(name="sps", bufs=2, space="PSUM"))

        # ---- constants ----
        gms = {}
        for key, d in gm_d.items():
            t = cpool.tile([W105, BLOCKS[key[1]][1]], dt.bfloat16, tag=f"gm{key}", name=f"gm_{key[0]}{key[1]}{key[2]}")
            nc.gpsimd.dma_start(out=t[:], in_=d[:])
            gms[key] = t
        tws = {}
        for key, d in tw_d.items():
            nm = "tw" + "".join(str(x) for x in key)
            rows = d.shape[0]
            t = cpool.tile([rows, W105], dt.bfloat16, tag=nm, name=nm)
            nc.gpsimd.dma_start(out=t[:], in_=d[:])
            tws[key] = t
        mks = {}
        for name, d in mk_d.items():
            t = cpool.tile([W105, 4], dt.bfloat16, tag=f"mk{name}", name=f"mk{name}")
            nc.gpsimd.dma_start(out=t[:], in_=d[:])
            mks[name] = t
        id128 = cpool.tile([128, 128], dt.float32, tag="id128")
        nc.gpsimd.dma_start(out=id128[:], in_=id128_d[:])
        id4 = cpool.tile([4, 4], dt.float32, tag="id4")
        nc.gpsimd.dma_start(out=id4[:], in_=id4_d[:])
        bias_t = cpool.tile([128, 1], dt.float32, tag="biasln")
        nc.vector.memset(bias_t[:], LNHALF)

        for c in range(NCHUNK):
            r0 = c * NB
            # ---- input DMA (single transfer each; host rows are p-major) ----
            nat = {}
            for comp, src in (("r", xhr), ("i", xhi)):
                t = natp.tile([128, NBLK * W105], dt.float32, tag=f"nat{comp}", name=f"nat{comp}")
                nc.sync.dma_start(out=t[:], in_=src[r0:r0 + NB, :])
                nat[comp] = t
            tit = natp.tile([128, NBLK * 4], dt.float32, tag="tit")
            nc.sync.dma_start(out=tit[:], in_=tih[r0:r0 + NB, :])

            # ---- E^T via PE transposes ----
            et = {}
            for comp in ("r", "i"):
                pt = wps.tile([W105, NB], dt.float32, tag="w", name=f"pt{comp}")
                for blk in range(NBLK):
                    nc.tensor.transpose(pt[:, blk * 128:(blk + 1) * 128],
                                        nat[comp][:, blk * W105:(blk + 1) * W105],
                                        id128[:])
                e = etp.tile([W105, NB], dt.bfloat16, tag=f"et{comp}", name=f"et{comp}")
                nc.scalar.copy(e[:], pt[:])
                et[comp] = e

            # ---- d0: |E|^2 rows (p-fold happens inside T weights) ----
            sqr = sqp.tile([W105, NB], dt.bfloat16, tag="sqr")
            nc.gpsimd.tensor_mul(sqr[:], et["r"][:], et["r"][:])
            sqi = sqp.tile([W105, NB], dt.bfloat16, tag="sqi")
            nc.gpsimd.tensor_mul(sqi[:], et["i"][:], et["i"][:])
            d0pre = cp.tile([W105, NB], dt.bfloat16, tag="d0pre")
            nc.gpsimd.tensor_add(d0pre[:], sqr[:], sqi[:])

            # ---- C blocks: gather stacks (p packed on free), products, fold ----
            cr = {}
            ci = {}
            for blk, (j0, jl) in enumerate(BLOCKS):
                sb = {}
                for st in ("A", "B"):
                    for comp in ("r", "i"):
                        t = stp.tile([128, 2 * NB], dt.bfloat16, tag=f"st{st}{comp}", name=f"st{st}{comp}")
                        for p in range(2):
                            ps = sps.tile([128, NB], dt.float32, tag="stps", name="stps")
                            for sl in range(NB // 512):
                                nc.tensor.matmul(
                                    ps[0:jl, sl * 512:(sl + 1) * 512],
                                    gms[(st, blk, p)][:],
                                    et[comp][:, sl * 512:(sl + 1) * 512],
                                    start=True, stop=True)
                            nc.scalar.copy(t[0:jl, p * NB:(p + 1) * NB], ps[0:jl, :])
                        sb[(st, comp)] = t
                m1 = prodp.tile([128, 2 * NB], dt.bfloat16, tag="m1")
                nc.vector.tensor_mul(m1[0:jl, :], sb[("A", "r")][0:jl, :], sb[("B", "r")][0:jl, :])
                m2 = prodp.tile([128, 2 * NB], dt.bfloat16, tag="m2")
                nc.vector.tensor_mul(m2[0:jl, :], sb[("A", "i")][0:jl, :], sb[("B", "i")][0:jl, :])
                m3 = prodp.tile([128, 2 * NB], dt.bfloat16, tag="m3")
                nc.vector.tensor_mul(m3[0:jl, :], sb[("A", "i")][0:jl, :], sb[("B", "r")][0:jl, :])
                m4 = prodp.tile([128, 2 * NB], dt.bfloat16, tag="m4")
                nc.vector.tensor_mul(m4[0:jl, :], sb[("A", "r")][0:jl, :], sb[("B", "i")][0:jl, :])
                crp = prep.tile([128, 2 * NB], dt.bfloat16, tag="crp")
                nc.vector.tensor_add(crp[0:jl, :], m1[0:jl, :], m2[0:jl, :])
                cip = prep.tile([128, 2 * NB], dt.bfloat16, tag="cip")
                nc.vector.tensor_sub(cip[0:jl, :], m3[0:jl, :], m4[0:jl, :])
                crf = cp.tile([128, NB], dt.bfloat16, tag="crf", name="crf")
                nc.vector.tensor_add(crf[0:jl, :], crp[0:jl, 0:NB], crp[0:jl, NB:2 * NB])
                cif = cp.tile([128, NB], dt.bfloat16, tag="cif", name="cif")
                nc.vector.tensor_add(cif[0:jl, :], cip[0:jl, 0:NB], cip[0:jl, NB:2 * NB])
                cr[blk] = crf
                ci[blk] = cif

            # ---- T matmuls (PSUM-accumulated weighted reduction over h) ----
            tsb = {}
            for tcomp in ("r", "i"):
                tps = wps.tile([W105, NB], dt.float32, tag="w", name=f"tps{tcomp}")
                for sl in range(NB // 512):
                    s0, s1 = sl * 512, (sl + 1) * 512
                    rhss = [
                        (tws[("Cr", 0, tcomp)], cr[0], BL0),
                        (tws[("Cr", 1, tcomp)], cr[1], BL1),
                        (tws[("Ci", 0, tcomp)], ci[0], BL0),
                        (tws[("Ci", 1, tcomp)], ci[1], BL1),
                        (tws[("D0", tcomp)], d0pre, W105),
                    ]
                    for k, (lt, rhs, rows) in enumerate(rhss):
                        nc.tensor.matmul(tps[:, s0:s1], lt[:],
                                         rhs[0:rows, s0:s1],
                                         start=(k == 0), stop=(k == len(rhss) - 1))
                t = tp.tile([W105, NB], dt.bfloat16, tag=f"tsb{tcomp}", name=f"tsb{tcomp}")
                nc.scalar.copy(t[:], tps[:])
                tsb[tcomp] = t

            # ---- X products and masked reduce ----
            px = {}
            for name, ec, tcp in (("P1", "r", "r"), ("P2", "i", "i"),
                                  ("P3", "r", "i"), ("P4", "i", "r")):
                t = xp.tile([W105, NB], dt.bfloat16, tag=f"px{name}", name=f"px{name}")
                nc.vector.tensor_mul(t[:], et[ec][:], tsb[tcp][:])
                px[name] = t
            eow = sps.tile([128, NB], dt.float32, tag="stps", name="eow")
            eops = eow[0:4, :]
            for sl in range(NB // 512):
                s0, s1 = sl * 512, (sl + 1) * 512
                for k, name in enumerate(("P1", "P2", "P3", "P4")):
                    nc.tensor.matmul(eops[:, s0:s1], mks[name][:], px[name][:, s0:s1],
                                     start=(k == 0), stop=(k == 3))
            eall = ep.tile([4, NB], dt.float32, tag="eall")
            nc.scalar.copy(eall[:], eops[:])

            # ---- finish: out = E_L + Eout * 10^(ti0/10)/2 ----
            pow_ = sps.tile([128, NB], dt.float32, tag="stps", name="powt")
            pops = pow_[:, 0:NBLK * 4]
            for blk in range(NBLK):
                nc.tensor.transpose(pops[:, blk * 4:(blk + 1) * 4],
                                    eall[:, blk * 128:(blk + 1) * 128], id4[:])
            import concourse.mybir as _mb
            pcol8 = finp.tile([128, NBLK], dt.float32, tag="pcol8")
            nc.scalar.activation(pcol8[:], tit[:, 0:NBLK * 4:4], _mb.ActivationFunctionType.Exp,
                                 bias=bias_t[:], scale=LN10_10)
            otmp = finp.tile([128, NBLK * 4], dt.float32, tag="otmp")
            for blk in range(NBLK):
                nc.vector.tensor_scalar_mul(otmp[:, blk * 4:(blk + 1) * 4],
                                            pops[:, blk * 4:(blk + 1) * 4],
                                            pcol8[:, blk:blk + 1])
            ot32 = finp.tile([128, NBLK * 4], dt.float32, tag="ot32")
            # cols (0,1,2,3)+4*blk += nat_{r,i} cols (20,20,84,84)+105*blk
            for cc, (comp, col) in enumerate((("r", 20), ("i", 20), ("r", 84), ("i", 84))):
                nc.vector.tensor_add(ot32[:, cc:NBLK * 4:4], otmp[:, cc:NBLK * 4:4],
                                     nat[comp][:, col:NBLK * W105:W105])
            nc.sync.dma_start(out=out_d[r0:r0 + NB, :], in_=ot32[:])

    nc.compile()
    return nc


_CACHE = {}


def _perm():
    """sample permutation: new row (chunk, p, blk) <- old row (chunk, blk, p)
    with p in [0,128), blk in [0,8) within each 1024-sample chunk."""
    s = np.arange(B).reshape(B // NB, NBLK, 128)      # [chunks, blk, p]
    return s.transpose(0, 2, 1).reshape(B)            # new-order -> old index


def kernel(xr, xi, task_info, Wr, Wi):
    from concourse.bass_utils import run_bass_kernel_spmd

    xr = np.asarray(xr, dtype=np.float32).reshape(B, M * NMODES)
    xi = np.asarray(xi, dtype=np.float32).reshape(B, M * NMODES)
    task_info = np.ascontiguousarray(np.asarray(task_info, dtype=np.float32))
    perm = _perm()
    xh = {}
    for comp, src in (("r", xr), ("i", xi)):
        t = np.zeros((B, W105), dtype=np.float32)
        sp = src[perm]
        t[:, 0:M] = sp[:, 0::2]        # p0: col 2k
        t[:, 64:64 + M] = sp[:, 1::2]  # p1: col 2k+1
        xh[comp] = t
    tih = task_info[perm]

    gm = _build_gmats()
    Wr32 = np.asarray(Wr, dtype=np.float32)
    Wi32 = np.asarray(Wi, dtype=np.float32)
    tw = _build_tw(Wr32, Wi32)
    mks = _build_masks()
    id128 = np.eye(128, dtype=np.float32)
    id4 = np.eye(4, dtype=np.float32)

    if "nc" not in _CACHE:
        _CACHE["nc"] = _build_kernel()
    nc = _CACHE["nc"]

    in_maps = []
    for core in range(NCORES):
        s = slice(core * BC, (core + 1) * BC)
        im = {"xhr": xh["r"][s], "xhi": xh["i"][s], "tih": tih[s],
              "id128": id128, "id4": id4}
        for key, v in gm.items():
            st, blk, p = key
            im[f"g{st}{blk}{p}"] = v
        for key, v in tw.items():
            im["tw" + "".join(str(x) for x in key)] = v
        for name, v in mks.items():
            im[f"mk{name}"] = v
        in_maps.append(im)
    res = run_bass_kernel_spmd(nc, in_maps, list(range(NCORES)))
    outs = [res.results[i]["out"] for i in range(NCORES)]
    full = np.concatenate(outs, axis=0)  # [B, 4] in permuted order
    inv = np.empty(B, dtype=np.int64)
    inv[perm] = np.arange(B)
    full = full[inv]
    return full.reshape(B, NMODES, 2).astype(np.float32)
